# revision 15
# baseline (speedup 1.0000x reference)
"""Multi-head attention (B=2, S=2048, D=1024, H=16, Dk=64) on 8 TRN2 cores.

Sharding: tensor-parallel on heads - 2 heads (dh=128 columns of the QKV
projections) per core.  Each core:
  1. projects qT/kT/vT = (W_slice.T @ x.T) for its 2 heads    [128, 4096]
  2. transposes vT into per-(b,h) [j, d] blocks with an appended
     ones-column (so P@V_aug also yields the softmax row-sums)
  3. scoresT = kT-stationary matmul -> pT = exp(scoresT/8) in [j, i]
     layout, PV accumulates oT_aug = [o_unnorm ; rowsums] in PSUM
  4. normalizes via PE-broadcast of 1/rowsum
  5. partialT = Wo_slice.T @ oT                               [1024, 4096]
Host sums the 8 partialT outputs, adds bo, and transposes back.

Engine assignment (keeps the PE saturated so it holds its top p-state):
  PE   - all matmuls/transposes (proj, scores, PV, 1/rowsum broadcast,
         out-projection)
  ACT  - exp ONLY, on [128,1024] tiles (2 PSUM banks per tile)
  DVE  - all PSUM evacuations (with fused bias add), reciprocal_approx,
         normalize multiply
  Pool - v_sb ones memset
The schedule software-pipelines scores ahead of PV and interleaves the
out-projection + finalize work into the attention passes as PE filler.
"""

from collections import deque

import numpy as np

D = 1024
NTOK = 4096  # B * S
B = 2
S = 2048
DH = 128  # head-dim block per core (2 heads x 64)
N_CORES = 8

_CACHE = {}


def _build_nc():
    import concourse.bacc as bacc
    import concourse.mybir as mybir
    import concourse.tile as tile

    dt = mybir.dt
    f32 = dt.float32
    f16 = dt.float16
    AF = mybir.ActivationFunctionType

    nc = bacc.Bacc("TRN2", target_bir_lowering=False, debug=False)

    xq = nc.dram_tensor("xq", [D, NTOK], f16, kind="ExternalInput").ap()
    xk = nc.dram_tensor("xk", [D, NTOK], f16, kind="ExternalInput").ap()
    xv = nc.dram_tensor("xv", [D, NTOK], f16, kind="ExternalInput").ap()
    xdram = [xq, xk, xv]
    wq = nc.dram_tensor("wq", [128, D], f16, kind="ExternalInput").ap()
    wk = nc.dram_tensor("wk", [128, D], f16, kind="ExternalInput").ap()
    wv = nc.dram_tensor("wv", [128, D], f16, kind="ExternalInput").ap()
    wo = nc.dram_tensor("wo", [128, D], f16, kind="ExternalInput").ap()
    bq = nc.dram_tensor("bq", [128, 1], f32, kind="ExternalInput").ap()
    bk = nc.dram_tensor("bk", [128, 1], f32, kind="ExternalInput").ap()
    bv = nc.dram_tensor("bv", [128, 1], f32, kind="ExternalInput").ap()
    c_ident = nc.dram_tensor("c_ident", [128, 64], f16, kind="ExternalInput").ap()
    c_ones64 = nc.dram_tensor("c_ones64", [1, 64], f16, kind="ExternalInput").ap()
    pout = nc.dram_tensor("pout", [D, NTOK], f16, kind="ExternalOutput").ap()

    with tile.TileContext(nc) as tc:
        from contextlib import ExitStack

        with ExitStack() as stk:
            const = stk.enter_context(tc.tile_pool(name="const", bufs=1))
            wpool = stk.enter_context(tc.tile_pool(name="w", bufs=1))
            big = stk.enter_context(tc.tile_pool(name="big", bufs=1))
            xpool = stk.enter_context(tc.tile_pool(name="xt", bufs=18))
            ptp = stk.enter_context(tc.tile_pool(name="pt", bufs=4))
            rsp = stk.enter_context(tc.tile_pool(name="rs", bufs=2))
            osp = stk.enter_context(tc.tile_pool(name="os", bufs=2))
            stp = stk.enter_context(tc.tile_pool(name="st", bufs=3))
            # PSUM: 'sc' 3 slots x 2 banks + 'ops' 1 slot x 2 banks = 8 banks
            psum = stk.enter_context(tc.tile_pool(name="ps", bufs=3, space="PSUM"))

            # ---- constants / weights (DMA first so ldweights can start) ----
            ident = const.tile([128, 64], f16)
            nc.sync.dma_start(out=ident, in_=c_ident)
            ones64 = const.tile([1, 64], f16)
            nc.sync.dma_start(out=ones64, in_=c_ones64)
            bq_sb = const.tile([128, 1], f32)
            bk_sb = const.tile([128, 1], f32)
            bv_sb = const.tile([128, 1], f32)
            nc.sync.dma_start(out=bq_sb, in_=bq)
            nc.sync.dma_start(out=bk_sb, in_=bk)
            nc.sync.dma_start(out=bv_sb, in_=bv)
            wq_sb = wpool.tile([128, D], f16)
            wk_sb = wpool.tile([128, D], f16)
            wv_sb = wpool.tile([128, D], f16)
            wo_sb = wpool.tile([128, D], f16)
            nc.sync.dma_start(out=wq_sb, in_=wq)

            # ---- persistent activations ----
            qT = big.tile([128, NTOK], f16)  # [dh, tok]
            kT = big.tile([128, NTOK], f16)
            oT = big.tile([128, NTOK], f16)  # [dh, tok] normalized context
            v_stage = big.tile([128, 2048], f16)  # per-batch vT staging
            v_sb = big.tile([128, 4 * 16 * 65], f16)  # [j, (b,h)*jt*(64+1)]
            nc.vector.memset(v_sb, 1.0)  # ones cols; v blocks overwritten
            v_r = v_sb.rearrange("p (t c) -> p t c", c=65)

            xtiles = {}

            def emit_xdma(ti, b, kk):
                x_t = xpool.tile([128, 2048], f16, tag="xt", name=f"x{ti}{b}_{kk}")
                nc.sync.dma_start(
                    out=x_t,
                    in_=xdram[ti][
                        kk * 128 : (kk + 1) * 128, b * 2048 : (b + 1) * 2048
                    ],
                )
                xtiles[(ti, b, kk)] = x_t

            # prologue: q weights + b0 x DMAs interleaved with remaining weights
            for kk in range(8):
                emit_xdma(0, 0, kk)
            nc.sync.dma_start(out=wk_sb, in_=wk)
            for kk in range(8):
                emit_xdma(1, 0, kk)
            nc.sync.dma_start(out=wv_sb, in_=wv)
            for kk in range(8):
                emit_xdma(2, 0, kk)
            nc.sync.dma_start(out=wo_sb, in_=wo)

            def gen_transp(b):
                """vT staging -> v_sb [j, d] blocks for batch b"""
                for h in range(2):
                    bh = b * 2 + h
                    tp = psum.tile([128, 1024], f16, tag="sc", name=f"tp{bh}")
                    for jt in range(16):
                        nc.tensor.transpose(
                            tp[:, jt * 64 : (jt + 1) * 64],
                            v_stage[h * 64 : (h + 1) * 64, jt * 128 : (jt + 1) * 128],
                            ident[h * 64 : (h + 1) * 64, :],
                        )
                    tp_r = tp.rearrange("p (t c) -> p t c", c=64)
                    nc.vector.tensor_copy(
                        v_r[:, bh * 16 : (bh + 1) * 16, 0:64], tp_r
                    )
                    yield

            def gen_proj(b, wide=True):
                """projections qT/kT/vT for batch b.

                wide=True: acc-pair tiles [128,1024] (2 nn chunks per PSUM
                slot, fewest evacs) - for dedicated proj phases.
                wide=False: [128,512] accs (1 bank, shorter slot holds) -
                for interleaving into attention passes as PE filler.
                """
                ncols = 1024 if wide else 512
                nacc = 2 if wide else 4
                for ti, (w_sb, b_sb, dst) in enumerate(
                    ((wq_sb, bq_sb, qT), (wk_sb, bk_sb, kT), (wv_sb, bv_sb, v_stage))
                ):
                    for ai in range(nacc):
                        acc = psum.tile(
                            [128, ncols], f32, tag="sc", name=f"acc{ti}{b}_{ai}"
                        )
                        for kk in range(8):
                            xt = xtiles[(ti, b, kk)]
                            for c in range(ncols // 512):
                                nn = ai * (ncols // 512) + c
                                nc.tensor.matmul(
                                    acc[:, c * 512 : (c + 1) * 512],
                                    lhsT=w_sb[:, kk * 128 : (kk + 1) * 128],
                                    rhs=xt[:, nn * 512 : (nn + 1) * 512],
                                    start=(kk == 0),
                                    stop=(kk == 7),
                                )
                        col = (b * 2048 if ti < 2 else 0) + ai * ncols
                        nc.vector.tensor_scalar_add(
                            dst[:, col : col + ncols], acc, b_sb
                        )
                        yield
                yield from gen_transp(b)

            def emit_sc(b, h, half, jt):
                """scores matmul pair + exp for one j-tile; returns pt"""
                i0 = b * 2048 + half * 1024
                sc = psum.tile(
                    [128, 1024], f32, tag="sc", name=f"s{b}{h}{half}_{jt}"
                )
                for c in range(2):
                    nc.tensor.matmul(
                        sc[:, c * 512 : (c + 1) * 512],
                        lhsT=kT[
                            h * 64 : (h + 1) * 64,
                            b * 2048 + jt * 128 : b * 2048 + (jt + 1) * 128,
                        ],
                        rhs=qT[
                            h * 64 : (h + 1) * 64, i0 + c * 512 : i0 + (c + 1) * 512
                        ],
                        start=True,
                        stop=True,
                    )
                pt = ptp.tile([128, 1024], f16, tag="pt", name=f"p{b}{h}{half}_{jt}")
                nc.scalar.activation(pt, sc, AF.Exp, scale=0.125)
                return pt

            def gen_finalize(o_st, b, h, half):
                """normalize: oT[h cols] = o_unnorm * broadcast(1/rowsum).

                Reads the SBUF evacuation (o_st) so the PSUM accumulator was
                already freed at pass end."""
                i0 = b * 2048 + half * 1024
                rinv16 = rsp.tile([1, 1024], f16, tag="ri16", name=f"rh{b}{h}{half}")
                with nc.allow_low_precision(reason="fp16 rinv is plenty"):
                    nc.vector.reciprocal(rinv16, o_st[64:65, :])
                yield
                Rp = psum.tile([64, 1024], f32, tag="sc", name=f"R{b}{h}{half}")
                for c in range(2):
                    nc.tensor.matmul(
                        Rp[:, c * 512 : (c + 1) * 512],
                        lhsT=ones64,
                        rhs=rinv16[:, c * 512 : (c + 1) * 512],
                        start=True,
                        stop=True,
                    )
                Rs = rsp.tile([64, 1024], f32, tag="rs", name=f"Rs{b}{h}{half}")
                nc.vector.tensor_copy(Rs, Rp)
                yield
                nc.vector.tensor_mul(
                    oT[h * 64 : (h + 1) * 64, i0 : i0 + 1024], o_st[0:64, :], Rs
                )
                yield

            def gen_outproj(b, half):
                """partialT[:, i-block] = Wo_c.T @ oT[:, i-block] -> DMA out"""
                i0 = b * 2048 + half * 1024
                for dt_ in range(8):
                    op = psum.tile(
                        [128, 1024], f32, tag="sc", name=f"op{b}{half}_{dt_}"
                    )
                    for c in range(2):
                        nc.tensor.matmul(
                            op[:, c * 512 : (c + 1) * 512],
                            lhsT=wo_sb[:, dt_ * 128 : (dt_ + 1) * 128],
                            rhs=oT[:, i0 + c * 512 : i0 + (c + 1) * 512],
                            start=True,
                            stop=True,
                        )
                    st = stp.tile([128, 1024], f16, tag="st", name=f"st{b}{half}_{dt_}")
                    nc.vector.tensor_copy(st, op)
                    nc.sync.dma_start(
                        out=pout[dt_ * 128 : (dt_ + 1) * 128, i0 : i0 + 1024],
                        in_=st,
                    )
                    yield

            # ---- filler management: PE/DVE work drip-fed into passes ----
            filler = deque()

            def pump(n):
                done = 0
                while filler and done < n:
                    try:
                        next(filler[0])
                        done += 1
                    except StopIteration:
                        filler.popleft()

            def drain():
                while filler:
                    pump(1_000_000)

            def gen_pass(b, h, half, fillers_per_iter=1):
                """one attention pass; scores pipelined one j-tile ahead.

                The single PSUM accumulator is evacuated to SBUF immediately
                at pass end (fast DVE copy) so the next pass can reuse the
                bank; normalization runs lazily from the SBUF copy."""
                bh = b * 2 + h
                o_ps = psum.tile(
                    [65, 1024], f32, tag="ops", bufs=1, name=f"o{bh}_{half}"
                )
                pts = deque()
                pts.append(emit_sc(b, h, half, 0))
                for jt in range(16):
                    if jt < 15:
                        pts.append(emit_sc(b, h, half, jt + 1))
                    pt = pts.popleft()
                    for c in range(2):
                        nc.tensor.matmul(
                            o_ps[:, c * 512 : (c + 1) * 512],
                            lhsT=v_sb[:, (bh * 16 + jt) * 65 : (bh * 16 + jt + 1) * 65],
                            rhs=pt[:, c * 512 : (c + 1) * 512],
                            start=(jt == 0),
                            stop=(jt == 15),
                        )
                    pump(fillers_per_iter)
                o_st = osp.tile([65, 1024], f32, tag="ost", name=f"oe{bh}_{half}")
                nc.vector.tensor_copy(o_st, o_ps)
                filler.appendleft(gen_finalize(o_st, b, h, half))

            # =========== emission schedule ===========
            # batch 0: proj + transposes (prologue, PE-serial, DMA-bound)
            for _ in gen_proj(0):
                pass

            # prefetch batch 1's x during batch 0's attention
            for ti in range(3):
                for kk in range(8):
                    emit_xdma(ti, 1, kk)

            gen_pass(0, 0, 0)
            gen_pass(0, 1, 0)
            # after both heads of half 0 finalize, out-project that half;
            # batch 1's projections ride along as PE filler
            filler.append(gen_outproj(0, 0))
            filler.append(gen_proj(1, wide=False))
            gen_pass(0, 0, 1)
            gen_pass(0, 1, 1)
            drain()
            filler.append(gen_outproj(0, 1))
            gen_pass(1, 0, 0)
            gen_pass(1, 1, 0)
            filler.append(gen_outproj(1, 0))
            gen_pass(1, 0, 1)
            gen_pass(1, 1, 1)
            drain()
            for _ in gen_outproj(1, 1):
                pass

    nc.compile()
    return nc


def _get_nc():
    key = "nc_v2"
    if key not in _CACHE:
        _CACHE[key] = _build_nc()
    return _CACHE[key]


def _ensure_ntff_hook():
    """Register the NTFF profile hook module if the image lacks it."""
    import sys
    import types

    if "antenv.axon_hooks" in sys.modules:
        return
    try:
        from trn_agent_boot.trn_boot import _ntff_profile_via_ctypes
    except Exception:
        return
    hook = None
    try:
        hook = _ntff_profile_via_ctypes("/opt/axon/libaxon_pjrt.so")
    except Exception:
        hook = None
    mod = types.ModuleType("antenv.axon_hooks")
    mod._hook = hook
    mod.get_axon_ntff_profile_hook = lambda: mod._hook
    mod.set_axon_ntff_profile_hook = lambda h: setattr(mod, "_hook", h)
    sys.modules["antenv.axon_hooks"] = mod


def _run(inputs, trace=False):
    from concourse import bass_utils

    if trace:
        _ensure_ntff_hook()

    nc = _get_nc()
    query = np.asarray(inputs["query"], np.float32)
    key = np.asarray(inputs["key"], np.float32)
    value = np.asarray(inputs["value"], np.float32)
    Wq = np.asarray(inputs["Wq"], np.float32)
    Wk = np.asarray(inputs["Wk"], np.float32)
    Wv = np.asarray(inputs["Wv"], np.float32)
    Wo = np.asarray(inputs["Wo"], np.float32)
    bq = np.asarray(inputs["bq"], np.float32)
    bk = np.asarray(inputs["bk"], np.float32)
    bv = np.asarray(inputs["bv"], np.float32)
    bo = np.asarray(inputs["bo"], np.float32)

    ext_dt = np.float16

    xqT = np.ascontiguousarray(query.reshape(NTOK, D).T.astype(ext_dt))
    xkT = np.ascontiguousarray(key.reshape(NTOK, D).T.astype(ext_dt))
    xvT = np.ascontiguousarray(value.reshape(NTOK, D).T.astype(ext_dt))

    def pack_w(Wc):
        return np.ascontiguousarray(
            Wc.reshape(8, 128, 128).transpose(1, 0, 2).reshape(128, D).astype(ext_dt)
        )

    ident_np = np.zeros((128, 64), np.float32)
    ident_np[np.arange(64), np.arange(64)] = 1.0
    ident_np[64 + np.arange(64), np.arange(64)] = 1.0
    consts = {
        "c_ident": np.ascontiguousarray(ident_np.astype(ext_dt)),
        "c_ones64": np.ones((1, 64), np.float16),
    }
    in_maps = []
    for c in range(N_CORES):
        sl = slice(c * 128, (c + 1) * 128)
        in_maps.append(
            {
                **consts,
                "xq": xqT,
                "xk": xkT,
                "xv": xvT,
                "wq": pack_w(Wq[:, sl]),
                "wk": pack_w(Wk[:, sl]),
                "wv": pack_w(Wv[:, sl]),
                "wo": np.ascontiguousarray(Wo[sl, :].astype(ext_dt)),
                "bq": np.ascontiguousarray(bq[sl].reshape(128, 1)),
                "bk": np.ascontiguousarray(bk[sl].reshape(128, 1)),
                "bv": np.ascontiguousarray(bv[sl].reshape(128, 1)),
            }
        )

    res = bass_utils.run_bass_kernel_spmd(
        nc, in_maps, core_ids=list(range(N_CORES)), trace=trace
    )
    outT = np.zeros((D, NTOK), np.float64)
    for c in range(N_CORES):
        outT += np.asarray(res.results[c]["pout"], np.float64)
    out = (outT.T + bo.astype(np.float64)).astype(np.float32)
    return out.reshape(B, S, D), res


def kernel(**inputs):
    out, _ = _run(inputs, trace=False)
    return out
